# revision 55
# baseline (speedup 1.0000x reference)
"""Trainium2 Bass kernel for the self-attention block (nn_Attention).

Reference computation (per batch b, row h):
    f = x @ wf + bf; g = x @ wg + bg; h = x @ wh + bh      (1x1 convs)
    s = g @ f^T (over W); beta = softmax(s, -1); o = beta @ h
    out = gamma * o + x

Sharding: data-parallel over batch B=8, one batch element per NeuronCore.
Per core, each of the 128 rows is an independent [W=128, C=512] block.

v2 dataflow (fp8 DoubleRow matmuls, row-pair batching):
  - xt is host-quantized to fp8e4 (|x| < 6 fits the e4m3 range); weights are
    host-scaled by 16 into fp8e4 so w*16 stays in the normal range. All
    de-scalings fold into existing instructions: the exp reads s via
    scale=1/256 and the softmax normalizer absorbs the 16x on h.
  - h/f/g projections run as fp8 DoubleRow matmuls (2 k-slices of 128 per
    instruction, 0.5 cycles/row): h is 2 matmuls per row instead of 4 bf16
    ones at 4x the cycles.
  - Rows are processed in pairs so the big PSUM->SBUF copies amortize their
    fixed overheads: one [128,2x512] h-copy (ScalarE), one [64,4x128]
    fg-copy (Pool/GPSIMD), one paired exp (ScalarE), one [128,2] reciprocal
    (DVE).
  - bh folds into the residual input exactly (sum_v beta=1), as gamma*bh
    added to x on the host. bf/bg are zero in this model; if nonzero they are
    applied via K=1 bias matmuls on the PE (build flag).
  - out = o*(gamma/(16Z)) + x fused in one DVE scalar_tensor_tensor per row.
  - DMA rings: x (residual, bf16) on SP, xt (fp8) on ACT, out (bf16) on DVE.
"""

import numpy as np
import ml_dtypes

import concourse.bacc as bacc
import concourse.bass as bass
import concourse.mybir as mybir
import concourse.tile as tile

B, H, W, C = 8, 128, 128, 512
CK = C // 8  # 64
N_CORES = 8
KT = C // 128  # 4 contraction slices
WS = 16.0  # host-side weight scale

F32 = mybir.dt.float32
BF16 = mybir.dt.bfloat16
FP8 = mybir.dt.float8e4
BFDT = ml_dtypes.bfloat16
F8DT = ml_dtypes.float8_e4m3
AF = mybir.ActivationFunctionType
ALU = mybir.AluOpType
DR = mybir.MatmulPerfMode.DoubleRow


def row_batch(rows: int) -> int:
    for rb in (8, 4, 2):
        if rows % rb == 0:
            return rb
    raise ValueError(f"rows={rows} must be even")


def build_nc(rows: int = H, fg_bias: bool = False) -> bass.Bass:
    nc = bacc.Bacc(None)
    RB = row_batch(rows)
    nrb = rows // RB
    x_d = nc.dram_tensor("x", [nrb, 128, RB * C], BF16, kind="ExternalInput")
    xt_d = nc.dram_tensor("xt", [nrb, 128, KT, RB * 128], FP8, kind="ExternalInput")
    wfg_d = nc.dram_tensor("wfg", [128, KT * 128], FP8, kind="ExternalInput")
    wh_d = nc.dram_tensor("wh", [128, KT * C], FP8, kind="ExternalInput")
    onesg_d = nc.dram_tensor("onesg", [W, 1], BF16, kind="ExternalInput")
    if fg_bias:
        bfg_d = nc.dram_tensor("bfg", [1, 128], BF16, kind="ExternalInput")
        onesr_d = nc.dram_tensor("onesr", [1, 256], BF16, kind="ExternalInput")
    out_d = nc.dram_tensor("out", [nrb, 128, RB * C], BF16, kind="ExternalOutput")

    with tile.TileContext(nc) as tc:
        with (
            tc.tile_pool(name="const", bufs=1) as cpool,
            tc.tile_pool(name="sb_x", bufs=6) as sb_x,
            tc.tile_pool(name="sb_xt", bufs=6) as sb_xt,
            tc.tile_pool(name="sb_h", bufs=3) as sb_h,
            tc.tile_pool(name="sb_fg", bufs=3) as sb_fg,
            tc.tile_pool(name="sb_at", bufs=3) as sb_at,
            tc.tile_pool(name="sb_out", bufs=5) as sb_out,
            tc.tile_pool(name="sb_small", bufs=6) as sb_small,
            tc.tile_pool(name="ps_h", bufs=2, space="PSUM") as ps_h,
            tc.tile_pool(name="ps_fg", bufs=1, space="PSUM") as ps_fg,
            tc.tile_pool(name="ps_os", bufs=3, space="PSUM") as ps_os,
        ):
            # weights ride the (initially idle) scalar/out queue so the first
            # xt/x input DMAs on sync aren't serialized behind them
            wh_sb = cpool.tile([128, KT, C], FP8)
            nc.scalar.dma_start(wh_sb[:, :, :], wh_d[:, :])
            wfg_sb = cpool.tile([128, KT, 128], FP8)
            nc.scalar.dma_start(wfg_sb[:, :, :], wfg_d[:, :])
            onesg_sb = cpool.tile([W, 1], BF16)
            nc.scalar.dma_start(onesg_sb[:], onesg_d[:])
            if fg_bias:
                bfg_sb = cpool.tile([1, 128], BF16)
                nc.scalar.dma_start(bfg_sb[:], bfg_d[:])
                onesr2_sb = cpool.tile([1, 256], BF16)
                nc.scalar.dma_start(onesr2_sb[:], onesr_d[:])

            # Dependency-free warm-up matmuls spin the PE during the DMA
            # bootstrap so the clock is at full p-state when real work lands.
            warm = cpool.tile([128, 128], BF16)
            nc.gpsimd.memset(warm[:, :], 0.0)
            warm_ps = ps_os.tile([128, 512], F32, tag="os")
            for _ in range(8):
                nc.tensor.matmul(
                    warm_ps[:, 0:128], lhsT=warm[:, :], rhs=warm[:, :],
                    start=True, stop=True,
                )

            # Software-pipelined over row pairs: Z/recip/o/STT of pair p run
            # during iteration p+1, filling the PE gap while ACT computes
            # fg-copy/exp for the current pair.
            HSPLIT = 384
            pend = None

            def finish_pend_tail():
                p = pend
                for r in range(2):
                    nc.tensor.matmul(
                        p["st"][:, 2, r : r + 1],
                        lhsT=p["at2"][:, r, :],
                        rhs=onesg_sb[:],
                        start=True,
                        stop=True,
                    )
                scale2 = sb_small.tile([128, 2], F32, tag="scale")
                nc.vector.reciprocal(scale2[:, :], p["st"][:, 2, 0:2])
                for r in range(2):
                    o_ps = ps_os.tile([128, 512], F32, tag="os")
                    nc.tensor.matmul(
                        o_ps[:, :],
                        lhsT=p["at2"][:, r, :],
                        rhs=p["h16"][:, r, :],
                        start=True,
                        stop=True,
                    )
                    nc.vector.scalar_tensor_tensor(
                        p["out4"][:, p["r0"] + r, :],
                        o_ps[:, :],
                        scale2[:, r : r + 1],
                        p["x4"][:, p["r0"] + r, :],
                        ALU.mult,
                        ALU.add,
                    )
                # outputs get their own HWDGE queue (scalar): no SWDGE
                # drains, and no head-of-line blocking of input prefetch
                if p["rb"] == nrb - 1:
                    # final block drains per-pair to shorten the tail
                    nc.scalar.dma_start(
                        out_d[p["rb"], :, p["r0"] * C : (p["r0"] + 2) * C],
                        p["out4"][:, p["r0"] : p["r0"] + 2, :],
                    )
                elif p["last_of_rb"]:
                    nc.scalar.dma_start(out_d[p["rb"]], p["out4"][:, :, :])

            for rb in range(nrb):
                # xt first: it feeds the first matmuls of the block
                xt4 = sb_xt.tile([128, KT, RB, 128], FP8, tag="xt8")
                if rb == 0:
                    # split so pair 0's rows land (and unblock the PE) early
                    nc.sync.dma_start(xt4[:, :, 0:2, :], xt_d[0, :, :, 0:256])
                    nc.sync.dma_start(
                        xt4[:, :, 2:RB, :], xt_d[0, :, :, 256 : RB * 128]
                    )
                else:
                    nc.sync.dma_start(xt4[:, :, :, :], xt_d[rb])
                x4 = sb_x.tile([128, RB, C], BF16, tag="x_row")
                nc.sync.dma_start(x4[:, :, :], x_d[rb])
                out4 = sb_out.tile([128, RB, C], BF16, tag="out_sb")
                for pr in range(RB // 2):
                    r0 = 2 * pr

                    # h[w,d] for both rows: fp8 DoubleRow, 2 k-pair matmuls/row.
                    # kp-outer order alternates PSUM banks between consecutive
                    # matmuls so same-bank accumulate turnaround is hidden.
                    h2 = ps_h.tile([128, 2, 512], F32, tag="h2")
                    for kp in range(2):
                        for r in range(2):
                            nc.tensor.matmul(
                                h2[:, r, :],
                                lhsT=xt4[:, 2 * kp : 2 * kp + 2, r0 + r, :],
                                rhs=wh_sb[:, 2 * kp : 2 * kp + 2, :],
                                start=(kp == 0),
                                stop=(kp == 1),
                                perf_mode=DR,
                            )
                    h16 = sb_h.tile([128, 2, 512], BF16, tag="h16")
                    # split the big h copy across ACT and DVE to balance load
                    nc.scalar.activation(
                        h16[:, :, 0:HSPLIT], h2[:, :, 0:HSPLIT], AF.Copy
                    )
                    nc.vector.tensor_copy(h16[:, :, HSPLIT:], h2[:, :, HSPLIT:])

                    # fT/gT [64, (rr,w)] for both rows in one matmul per k-pair
                    fg2 = ps_fg.tile([64, 2, 256], F32, tag="fg")
                    for half, woff in ((0, 0), (1, CK)):
                        if fg_bias:
                            nc.tensor.matmul(
                                fg2[:, half, :],
                                lhsT=bfg_sb[:, woff : woff + CK],
                                rhs=onesr2_sb[:, :],
                                start=True,
                                stop=False,
                            )
                        for kp in range(2):
                            nc.tensor.matmul(
                                fg2[:, half, :],
                                lhsT=wfg_sb[:, 2 * kp : 2 * kp + 2, woff : woff + CK],
                                rhs=xt4[:, 2 * kp : 2 * kp + 2, r0 : r0 + 2, :],
                                start=(kp == 0 and not fg_bias),
                                stop=(kp == 1),
                                perf_mode=DR,
                            )
                    fg16 = sb_fg.tile([64, 2, 256], BF16, tag="fg16")
                    nc.scalar.activation(fg16[:, :, :], fg2[:, :, :], AF.Copy)

                    # deferred tail of the previous pair fills the PE bubble
                    # while ACT finishes fg-copy and exp of this pair
                    if pend is not None:
                        finish_pend_tail()

                    # sT[v,w] per row (256x scaled); Z slots in [:, 2, 0:2]
                    st = ps_os.tile([128, 3, 128], F32, tag="os")
                    for r in range(2):
                        nc.tensor.matmul(
                            st[:, r, :],
                            lhsT=fg16[:, 0, r * 128 : (r + 1) * 128],
                            rhs=fg16[:, 1, r * 128 : (r + 1) * 128],
                            start=True, stop=True,
                        )
                    at2 = sb_at.tile([128, 2, 128], BF16, tag="at2")
                    nc.scalar.activation(
                        at2[:, :, :], st[:, 0:2, :], AF.Exp, scale=1.0 / (WS * WS)
                    )
                    pend = {
                        "st": st,
                        "at2": at2,
                        "h16": h16,
                        "x4": x4,
                        "out4": out4,
                        "r0": r0,
                        "rb": rb,
                        "last_of_rb": pr == RB // 2 - 1,
                    }
            finish_pend_tail()
    nc.compile()
    return nc


def make_in_map(x_b: np.ndarray, wf, bf, wg, bg, wh, bh, gamma) -> dict:
    x_b = np.asarray(x_b, np.float32)
    rows = x_b.shape[0]
    RB = row_batch(rows)
    nrb = rows // RB
    gamma_f = float(np.float32(np.asarray(gamma)))
    bh = np.asarray(bh, np.float32)
    # residual with gamma*bh folded in exactly (sum_v beta = 1)
    xr = x_b + gamma_f * bh  # [rows, W, C]
    x4 = np.ascontiguousarray(
        xr.astype(BFDT)
        .reshape(nrb, RB, W, C)
        .transpose(0, 2, 1, 3)
        .reshape(nrb, 128, RB * C)
    )
    # pre-transposed fp8 x: [rb, p(chan within slice), k, rr, w] (k-outer)
    xt = np.ascontiguousarray(
        x_b.astype(F8DT)
        .reshape(nrb, RB, W, KT, 128)
        .transpose(0, 4, 3, 1, 2)
        .reshape(nrb, 128, KT, RB * 128)
    )
    # weights: 16x scaled fp8, [p, k, m] layouts
    wfg = np.concatenate([np.asarray(wf), np.asarray(wg)], axis=1)  # [C, 128]
    wfg8 = np.ascontiguousarray(
        (wfg * WS).astype(F8DT).reshape(KT, 128, 128).transpose(1, 0, 2).reshape(128, KT * 128)
    )
    wh8 = np.ascontiguousarray(
        (np.asarray(wh) * WS).astype(F8DT).reshape(KT, 128, C).transpose(1, 0, 2).reshape(128, KT * C)
    )
    onesg = np.full((W, 1), WS / gamma_f, np.float32).astype(BFDT)
    m = {
        "x": x4,
        "xt": xt,
        "wfg": wfg8,
        "wh": wh8,
        "onesg": onesg,
    }
    bf = np.asarray(bf, np.float32)
    bg = np.asarray(bg, np.float32)
    if np.any(bf != 0) or np.any(bg != 0):
        m["bfg"] = np.concatenate([bf, bg]).reshape(1, 128).astype(BFDT)
        m["onesr"] = np.ones((1, 256), np.float32).astype(BFDT)
    return m


def unbatch_out(arr: np.ndarray, rows: int) -> np.ndarray:
    """[nrb, 128, RB*C] device layout -> [rows, W, C] f32."""
    RB = row_batch(rows)
    nrb = rows // RB
    return (
        np.asarray(arr)
        .astype(np.float32)
        .reshape(nrb, 128, RB, C)
        .transpose(0, 2, 1, 3)
        .reshape(rows, W, C)
    )


_NC_CACHE: dict = {}


def run(inputs: dict, trace: bool = False, **run_kwargs):
    """Build (cached), run on 8 cores, return (out, BassKernelResults)."""
    from concourse.bass_utils import run_bass_kernel_spmd

    gamma_f = float(np.float32(np.asarray(inputs["gamma"])))
    if gamma_f == 0.0:
        out = np.asarray(inputs["x"], np.float32).copy()
        return out, None
    fg_bias = bool(
        np.any(np.asarray(inputs["bf"]) != 0) or np.any(np.asarray(inputs["bg"]) != 0)
    )
    key = (H, fg_bias)
    if key not in _NC_CACHE:
        _NC_CACHE[key] = build_nc(H, fg_bias=fg_bias)
    nc = _NC_CACHE[key]
    x = np.asarray(inputs["x"], np.float32)
    in_maps = [
        make_in_map(
            x[b],
            inputs["wf"],
            inputs["bf"],
            inputs["wg"],
            inputs["bg"],
            inputs["wh"],
            inputs["bh"],
            inputs["gamma"],
        )
        for b in range(N_CORES)
    ]
    res = run_bass_kernel_spmd(
        nc, in_maps, list(range(N_CORES)), trace=trace, **run_kwargs
    )
    out = np.stack(
        [unbatch_out(res.results[b]["out"], H) for b in range(N_CORES)], axis=0
    )
    return out, res


def kernel(**inputs) -> np.ndarray:
    out, _ = run(inputs, trace=False)
    return out
